# revision 25
# baseline (speedup 1.0000x reference)
"""Multi-head causal attention (B=2, S=2048, D=1024, H=16, Dh=64) on 8 trn2 cores.

Sharding: data-parallel over batch (2) x tensor-parallel over head groups (4):
core c handles batch c//4, heads [4*(c%4), 4*(c%4)+4).

The reference reshapes ctx [B,H,S,Dh] -> [B,S,D] WITHOUT transposing heads
back, so output rows [128h, 128h+128) of batch b depend only on head h:
  out[b, 128h + r, :] = ctx_flat[r, :] @ wo,
  ctx_flat[r, s_lo*64 + dh] = ctx[b, h, 16r + s_lo, dh].
Each core therefore computes 4 independent 128-row output blocks; no
cross-core communication is needed.

Device kernel per core (fp16 matmul operands, fp32 PSUM accumulation):
  QT/KT [128(2 heads), 2048] = w.T @ x.T  (transposed projections)
  V     [128, 16, 4, 65] with ones column (denominator trick)
  scoresT[k, q] = K @ Q.T per 128k x 512q block, causal-skipped,
  exp on ScalarE (scores bounded, no max subtraction), causal fill via
  gpsimd affine_select, PV: ctxT_aug[65, q] += V_aug.T @ attnT
  (psum row 64 = softmax denominator),
  normalize: K=1 matmul broadcasts denom to partitions 0-63,
  reciprocal_approx_fast, multiply into ctx [64, 16, 128],
  out-proj: 16 K=64 matmuls vs wo [64, 2, 1024] tiles.

Scheduling: engines are in-order, so the trace interleaves PE "filler"
work (second head-pair's projections, first pair's output projection)
into the ACT-bound attention inner loop, and PV matmuls trail the
score matmuls by one k-block so the PE never stalls on the exp.
"""

import numpy as np


import concourse.bass as bass
import concourse.mybir as mybir
import concourse.tile as tile
from concourse import bacc
from concourse.bass_utils import run_bass_kernel_spmd

F16 = mybir.dt.float16
F32 = mybir.dt.float32

B, S, D, H, DH = 2, 2048, 1024, 16, 64
P = 128
HPC = 4          # heads per core
NCORES = 8
NQC = 4          # q chunks of 512
SCALE = 1.0 / 8.0

_CACHED_NC = None


def build_nc(debug=False):
    nc = bacc.Bacc(None)
    xt = nc.dram_tensor("xt", [D, S], F16, kind="ExternalInput")
    wq = nc.dram_tensor("wq", [D, HPC * DH], F16, kind="ExternalInput")
    wk = nc.dram_tensor("wk", [D, HPC * DH], F16, kind="ExternalInput")
    wv = nc.dram_tensor("wv", [D, HPC * DH], F16, kind="ExternalInput")
    wo = nc.dram_tensor("wo", [D, D], F16, kind="ExternalInput")
    out = nc.dram_tensor("out", [HPC * P, D], F16, kind="ExternalOutput")
    dbg = {}
    if debug:
        dbg["qt0"] = nc.dram_tensor("qt0", [P, S], F16, kind="ExternalOutput")
        dbg["kt0"] = nc.dram_tensor("kt0", [P, S], F16, kind="ExternalOutput")
        dbg["v0"] = nc.dram_tensor("v0", [P, 16, HPC * 65], F16, kind="ExternalOutput")
        dbg["ctx0"] = nc.dram_tensor("ctx0", [P, 8, P], F16, kind="ExternalOutput")
        dbg["a00"] = nc.dram_tensor("a00", [P, 2, 512], F16, kind="ExternalOutput")

    with tile.TileContext(nc) as tc:
        with (
            tc.tile_pool(name="big", bufs=8) as big,
            tc.tile_pool(name="wp", bufs=1) as wp,
            tc.tile_pool(name="qk", bufs=2) as qk,
            tc.tile_pool(name="vp", bufs=1) as vp,
            tc.tile_pool(name="apool", bufs=6) as apool,
            tc.tile_pool(name="cr", bufs=4) as cr,
            tc.tile_pool(name="cx", bufs=4) as cx,
            tc.tile_pool(name="ob", bufs=2) as ob,
            tc.tile_pool(name="cst", bufs=1) as cst,
            tc.tile_pool(name="psS", bufs=2, space="PSUM") as psS,
            tc.tile_pool(name="psV", bufs=2, space="PSUM") as psV,
            tc.tile_pool(name="psP", bufs=2, space="PSUM") as psP,
        ):
            # ---- loads ----
            w_sb = {}
            for name, w in (("q", wq), ("k", wk), ("v", wv)):
                t = wp.tile([P, 8, HPC * DH], F16, tag=f"w{name}", name=f"w_{name}")
                nc.sync.dma_start(t[:], w.rearrange("(dc p) n -> p dc n", p=P))
                w_sb[name] = t

            xt_r = xt.rearrange("(dc p) s -> dc p s", p=P)  # [8, 128, S]
            xt_sb = []
            for dc in range(8):
                t = big.tile([P, S], F16, tag="big", name=f"xt_{dc}")
                nc.sync.dma_start(t[:], xt_r[dc])
                xt_sb.append(t)

            ones_sb = cst.tile([P, 64], F16, tag="ones")
            nc.gpsimd.memset(ones_sb[:], 1.0)
            from concourse.masks import make_upper_triangular
            mask = cst.tile([P, P], F16, tag="mask")
            make_upper_triangular(nc, mask[:], val=1.0, diag=True)

            QT = {}
            KT = {}
            for pair in range(2):
                QT[pair] = qk.tile([P, S], F16, tag="qt", name=f"qt_{pair}")
                KT[pair] = qk.tile([P, S], F16, tag="kt", name=f"kt_{pair}")
            v_sb = vp.tile([P, 16, HPC, 65], F16, tag="v")
            nc.gpsimd.memset(v_sb[:, :, :, 64:65], 1.0)

            wo_sb = wp.tile([P, 8, D], F16, tag="wo", name="wo_sb")
            nc.sync.dma_start(
                wo_sb[:], wo.rearrange("(t p) n -> p t n", p=P)
            )

            # ---- emitters ----
            def emit_qk_chunk(name, pair, qc):
                t = QT[pair] if name == "q" else KT[pair]
                ps = psP.tile([P, 512], F32, tag="pj", name=f"pj_{name}{pair}{qc}")
                for dc in range(8):
                    nc.tensor.matmul(
                        ps[:],
                        w_sb[name][:, dc, P * pair : P * (pair + 1)],
                        xt_sb[dc][:, 512 * qc : 512 * (qc + 1)],
                        start=(dc == 0),
                        stop=(dc == 7),
                    )
                nc.vector.tensor_copy(t[:, 512 * qc : 512 * (qc + 1)], ps[:])

            def emit_v_block(sb):
                ps = psP.tile([P, 512], F32, tag="pj", name=f"pj_v{sb}")
                for dc in range(8):
                    nc.tensor.matmul(
                        ps[:, : HPC * DH],
                        xt_sb[dc][:, P * sb : P * (sb + 1)],
                        w_sb["v"][:, dc, :],
                        start=(dc == 0),
                        stop=(dc == 7),
                    )
                nc.vector.tensor_copy(
                    v_sb[:, sb, :, 0:64],
                    ps[:, : HPC * DH].rearrange("p (h d) -> p h d", h=HPC),
                )

            def emit_outproj(pair, h2, ctx_t):
                osb = ob.tile([P, D], F16, tag="ob", name=f"ob_{pair}_{h2}")
                for ncg in range(2):
                    pso = psP.tile([P, 512], F32, tag="pj", name=f"pjo_{pair}{h2}{ncg}")
                    for t in range(8):
                        nc.tensor.matmul(
                            pso[:],
                            ctx_t[:, t, :],
                            wo_sb[:, t, 512 * ncg : 512 * (ncg + 1)],
                            start=(t == 0),
                            stop=(t == 7),
                        )
                    nc.vector.tensor_copy(osb[:, 512 * ncg : 512 * (ncg + 1)], pso[:])
                    hl = 2 * pair + h2
                    nc.sync.dma_start(
                        out[P * hl : P * (hl + 1), 512 * ncg : 512 * (ncg + 1)],
                        osb[:, 512 * ncg : 512 * (ncg + 1)],
                    )

            # ---- attention inner pieces ----
            def emit_scores(pair, qc, kblk, off):
                sp = psS.tile([P, 2, 512], F32, tag="sc", name=f"sc_{pair}_{qc}_{kblk}")
                for h2 in range(2):
                    nc.tensor.matmul(
                        sp[:, h2, off:512],
                        KT[pair][64 * h2 : 64 * h2 + 64, P * kblk : P * (kblk + 1)],
                        QT[pair][64 * h2 : 64 * h2 + 64, 512 * qc + off : 512 * (qc + 1)],
                        start=True,
                        stop=True,
                        tile_position=(64 * h2, 0),
                    )
                return sp

            def emit_exp_mask(pair, qc, kblk, off, sp):
                a = apool.tile([P, 2, 512], F16, tag="a", name=f"a_{pair}_{qc}_{kblk}")
                nc.scalar.activation(
                    out=a[:, :, off:512],
                    in_=sp[:, :, off:512],
                    func=mybir.ActivationFunctionType.Exp,
                    scale=SCALE,
                )
                if kblk >= 4 * qc:
                    for h2 in range(2):
                        nc.vector.tensor_mul(
                            a[:, h2, off : off + P],
                            a[:, h2, off : off + P],
                            mask[:],
                        )
                return a

            def emit_pv(pair, kblk, off, a, pvs, nkb):
                for h2 in range(2):
                    nc.tensor.matmul(
                        pvs[h2][0:65, off:512],
                        v_sb[:, kblk, 2 * pair + h2, :],
                        a[:, h2, off:512],
                        start=(kblk == 0),
                        stop=(kblk == nkb - 1),
                    )

            def emit_normalize(pair, qc, h2, pvs, ctx_t):
                craw = cr.tile([65, 512], F16, tag="craw", name=f"cr_{pair}{qc}{h2}")
                nc.vector.tensor_copy(craw[:], pvs[h2][0:65, :])
                bcp = psP.tile([P, 512], F32, tag="pj", name=f"bc_{pair}_{qc}_{h2}")
                nc.tensor.matmul(
                    bcp[0:64, :],
                    ones_sb[64:65, 0:64],
                    craw[64:65, :],
                    start=True,
                    stop=True,
                )
                denr = cr.tile([64, 512], F32, tag="denr", name=f"dr_{pair}{qc}{h2}")
                nc.vector.reciprocal_approx_fast(out=denr[:], in_=bcp[0:64, :])
                srcv = craw[0:64, :].rearrange("d (r t2) -> d t2 r", t2=16)
                recv = denr[:, :].rearrange("d (r t2) -> d t2 r", t2=16)
                for par in range(2):
                    dstv = ctx_t[
                        64 * par : 64 * (par + 1), :, 32 * qc : 32 * (qc + 1)
                    ]
                    nc.gpsimd.tensor_mul(
                        dstv, srcv[:, par::2, :], recv[:, par::2, :]
                    )
                if debug and pair == 0 and qc == 3 and h2 == 0:
                    nc.sync.dma_start(dbg["ctx0"][:], ctx_t[:])

            # ---- phase A: pair-0 projections + first half of V ----
            for qc in range(NQC):
                emit_qk_chunk("k", 0, qc)
                emit_qk_chunk("q", 0, qc)
            for sb in range(8):
                emit_v_block(sb)

            # ---- filler streams (consumed between attention groups) ----
            def filler_stream_b():
                for sb in range(8, 16):
                    yield lambda sb=sb: emit_v_block(sb)
                for qc in range(NQC):
                    yield lambda qc=qc: emit_qk_chunk("k", 1, qc)
                    yield lambda qc=qc: emit_qk_chunk("q", 1, qc)

            def filler_stream_c():
                yield lambda: emit_outproj(0, 0, ctx_tiles[0][0])
                yield lambda: emit_outproj(0, 1, ctx_tiles[0][1])

            # ---- attention for a pair, with filler interleave ----
            ctx_tiles = {}

            def emit_attention(pair, filler, per_group):
                ctxs = [
                    cx.tile([P, 8, P], F16, tag="cx", name=f"ctx_{pair}_{i}")
                    for i in range(2)
                ]
                ctx_tiles[pair] = ctxs
                budget = 0.0
                done = False
                for qc in range(NQC):
                    pvs = [
                        psV.tile([P, 512], F32, tag="pv", name=f"pv_{pair}_{qc}_{i}")
                        for i in range(2)
                    ]
                    nkb = 4 * qc + 4
                    pending = []
                    for kblk in range(nkb):
                        off = max(0, P * kblk - 512 * qc)
                        sp = emit_scores(pair, qc, kblk, off)
                        if len(pending) >= 2:
                            pr = pending.pop(0)
                            emit_pv(pair, pr[1], pr[2], pr[0], pvs, nkb)
                        a = emit_exp_mask(pair, qc, kblk, off, sp)
                        if debug and pair == 0 and qc == 0 and kblk == 0:
                            nc.sync.dma_start(dbg["a00"][:], a[:])
                        pending.append((a, kblk, off))
                        if not done:
                            budget += per_group
                            while budget >= 1.0:
                                budget -= 1.0
                                try:
                                    next(filler)()
                                except StopIteration:
                                    done = True
                                    break
                    for pr in pending:
                        emit_pv(pair, pr[1], pr[2], pr[0], pvs, nkb)
                    for h2 in range(2):
                        emit_normalize(pair, qc, h2, pvs, ctxs[h2])
                # drain filler
                while not done:
                    try:
                        next(filler)()
                    except StopIteration:
                        done = True

            emit_attention(0, filler_stream_b(), per_group=1.0)

            if debug:
                nc.sync.dma_start(
                    dbg["v0"][:], v_sb[:].rearrange("p s h c -> p s (h c)")
                )
                nc.sync.dma_start(dbg["qt0"][:], QT[0][:])
                nc.sync.dma_start(dbg["kt0"][:], KT[0][:])

            emit_attention(1, filler_stream_c(), per_group=0.05)

            emit_outproj(1, 0, ctx_tiles[1][0])
            emit_outproj(1, 1, ctx_tiles[1][1])

    nc.compile()
    return nc


def make_in_maps(x, wq, wk, wv, wo):
    in_maps = []
    for c in range(NCORES):
        b, g = divmod(c, HPC)
        cols = slice(g * HPC * DH, (g + 1) * HPC * DH)
        in_maps.append(
            {
                "xt": np.ascontiguousarray(x[b].T).astype(np.float16),
                "wq": np.ascontiguousarray(wq[:, cols]).astype(np.float16),
                "wk": np.ascontiguousarray(wk[:, cols]).astype(np.float16),
                "wv": np.ascontiguousarray(wv[:, cols]).astype(np.float16),
                "wo": np.ascontiguousarray(wo).astype(np.float16),
            }
        )
    return in_maps


def gather_out(results):
    out = np.empty((B, S, D), np.float32)
    for c in range(NCORES):
        b, g = divmod(c, HPC)
        o = results[c]["out"]
        for j in range(HPC):
            h = g * HPC + j
            out[b, P * h : P * (h + 1), :] = o[P * j : P * (j + 1), :]
    return out


def kernel(x, wq, wk, wv, wo):
    global _CACHED_NC
    if _CACHED_NC is None:
        _CACHED_NC = build_nc()
    x = np.asarray(x, dtype=np.float32)
    res = run_bass_kernel_spmd(
        _CACHED_NC,
        make_in_maps(
            x,
            np.asarray(wq, np.float32),
            np.asarray(wk, np.float32),
            np.asarray(wv, np.float32),
            np.asarray(wo, np.float32),
        ),
        core_ids=list(range(NCORES)),
    )
    return gather_out(res.results)
